# revision 25
# baseline (speedup 1.0000x reference)
"""EntropyGuidedAttention TRN2 kernel.

Head-sharded across 8 NeuronCores (2 heads/core). Per (head, query-row) the
reference keeps the top-k_keep attention scores (k from the frozen entropy
EMA/threshold), renormalizes, applies V and the output projection.

Device algorithm per head, per 128-query tile (scores laid [q_part, key_free]):
  - scores via PE matmuls from QT/KT (both computed on-device from xT)
  - N = 16 - s  (negated-shifted scores; all selection logic runs on N,
    "keep" == N <= t; N > 13 always so masked-multiply tricks stay sign-safe)
  - per-row Gaussian init (bn_stats on a 512-col subsample) then a 5-probe
    secant/bisection ladder on exact fused count passes
    (tensor_scalar is_le + accum_out) landing on the smallest over-count
  - exact snap: masked max8 gives the 8 smallest kept scores; a one-hot
    select of u[excess] moves the threshold to the exact k-th boundary value
    (no value round-trips, so the final mask count is exact for excess <= 7)
  - w = (N <= t_fin) * exp(s) with the row-sum Sk accumulated in the same op
  - w^T with 1/Sk folded in via a matmul against diag(1/Sk)  (transpose +
    renormalize in one PE pass), then AV accumulation -> per-head O^T
  - output projection vs the core's 128 feature rows of Wo^T
Host: computes k_keep from entropy inputs, transposes/slices weights, sums the
8 partial outputs (the Megatron row-parallel all-reduce) and adds the bias.
"""

import os
import numpy as np
from statistics import NormalDist
_STAGE = float(os.environ.get("KERNEL_STAGE", "3"))

D_MODEL = 1024
N_HEADS = 16
D_HEAD = 64
L = 2048
MIN_SPARSITY = 0.1
NCORES = 8
QT = L // 128  # 16 query tiles per head

_BUILD_CACHE = {}


def _build_nc():
    import concourse.mybir as mybir
    import concourse.tile as tile
    from concourse import bacc

    f32 = mybir.dt.float32
    f16 = mybir.dt.float16
    Alu = mybir.AluOpType
    Act = mybir.ActivationFunctionType

    nc = bacc.Bacc(None, target_bir_lowering=False, debug=False, num_devices=NCORES)

    LSH = L // NCORES  # 256-column xT shard per core
    # flat f16 pack: xs | wqT | wkT | wvT | woT  (element offsets)
    O_XS, N_XS = 0, D_MODEL * LSH
    O_WQ = O_XS + N_XS
    O_WK = O_WQ + D_MODEL * 128
    O_WV = O_WK + D_MODEL * 128
    O_WO = O_WV + D_MODEL * 128
    N_PK = O_WO + 128 * D_MODEL
    pk_d = nc.declare_dram_parameter("pk16", [N_PK], f16, isOutput=False)
    pc_d = nc.declare_dram_parameter("pk32", [128, 18], f32, isOutput=False)
    out_d = nc.declare_dram_parameter("outp", [LSH, D_MODEL], f16, isOutput=True)
    pk_ap = pk_d.ap()

    with tile.TileContext(nc) as tc:
        with (
            tc.tile_pool(name="const", bufs=1) as cpool,
            tc.tile_pool(name="persist", bufs=1) as ppool,
            tc.tile_pool(name="state", bufs=1) as spool,
            tc.tile_pool(name="big", bufs=2) as bigpool,
            tc.tile_pool(name="wt", bufs=2) as wtpool,
            tc.tile_pool(name="small", bufs=2) as smpool,
            tc.tile_pool(name="dram", bufs=1, space="DRAM") as dpool,
        ):
            # ---- x AllGather: each core ships 1/8 of xT (f16), gather on-device ----
            xin_b = dpool.tile([D_MODEL, LSH], f16, tag="xin")
            nc.sync.dma_start(
                xin_b[:], pk_ap[O_XS:O_XS + N_XS].rearrange("(d ll) -> d ll", ll=LSH)
            )
            xg_b = dpool.tile([NCORES * D_MODEL, LSH], f16, tag="xg")
            nc.gpsimd.collective_compute(
                "AllGather",
                mybir.AluOpType.bypass,
                replica_groups=[list(range(NCORES))],
                ins=[xin_b.opt()],
                outs=[xg_b.opt()],
            )
            # partial y for the row-parallel sum; ReduceScatter target
            yb_b = dpool.tile([L, D_MODEL], f32, tag="yb")
            ys_b = dpool.tile([LSH, D_MODEL], f32, tag="ys")

            # ---- constant loads ----
            wof_sb = cpool.tile([64, 2, D_MODEL], f16, tag="wof")
            nc.sync.dma_start(
                wof_sb[:],
                pk_ap[O_WO:O_WO + 128 * D_MODEL].rearrange("(h p m) -> p h m", p=64, m=D_MODEL),
            )
            wo_sb = cpool.tile([64, 2, D_MODEL], f32, tag="wo")
            nc.scalar.activation(wo_sb[:], wof_sb[:], Act.Identity)
            bq_sb = cpool.tile([64, 2], f32, tag="bq")
            nc.gpsimd.dma_start(bq_sb[:], pc_d.ap()[:, 0:1].rearrange("(h p) o -> p (h o)", p=64))
            bk_sb = cpool.tile([64, 2], f32, tag="bk")
            nc.gpsimd.dma_start(bk_sb[:], pc_d.ap()[:, 1:2].rearrange("(h p) o -> p (h o)", p=64))
            hc_sb = cpool.tile([128, 2, 8], f32, tag="hc")
            nc.gpsimd.dma_start(hc_sb[:], pc_d.ap()[:, 2:18].rearrange("p (h e) -> p h e", e=8))
            io8_sb = cpool.tile([128, 8], f32, tag="io8")
            nc.gpsimd.iota(io8_sb[:], pattern=[[1, 8]], base=0, channel_multiplier=0,
                           allow_small_or_imprecise_dtypes=True)
            io128_sb = cpool.tile([128, 128], f32, tag="io128")
            nc.gpsimd.iota(io128_sb[:], pattern=[[1, 128]], base=0, channel_multiplier=0,
                           allow_small_or_imprecise_dtypes=True)
            pid_sb = cpool.tile([128, 1], f32, tag="pid")
            nc.gpsimd.iota(pid_sb[:], pattern=[[0, 1]], base=0, channel_multiplier=1,
                           allow_small_or_imprecise_dtypes=True)
            b16_sb = cpool.tile([128, 1], f32, tag="b16")
            nc.vector.memset(b16_sb[:], 16.0)

            # ---- persistent intermediates ----
            qT_sb = ppool.tile([64, 2, L], f32, tag="qT")   # [dh, head, q]
            kT_sb = ppool.tile([64, 2, L], f32, tag="kT")   # [dh, head, k]
            v_sb = ppool.tile([128, QT, 128], f32, tag="v")  # [k_in_tile, ktile, (h,dh)]
            yT_sb = ppool.tile([64, 2, QT, 128], f32, tag="yT")  # [dh, head, qtile, q]

            # ---- phase A: projections (xT + W tiles scoped to this phase) ----
            with (
                tc.tile_pool(name="xw", bufs=1) as xwpool,
                tc.tile_pool(name="pA", bufs=2, space="PSUM") as pA,
            ):
                xT_sb = xwpool.tile([128, 8, L], f16, tag="xT")
                # xg_b[b*1024 + dc*128 + p, ll] -> xT_sb[p, dc, b*256+ll]
                for b in range(NCORES):
                    nc.gpsimd.dma_start(
                        xT_sb[:, :, b * LSH:(b + 1) * LSH],
                        xg_b[b * D_MODEL:(b + 1) * D_MODEL, :].rearrange(
                            "(dc p) ll -> p dc ll", p=128
                        ),
                    )
                wq_sb = xwpool.tile([128, 8, 128], f16, tag="wq")
                nc.scalar.dma_start(
                    wq_sb[:],
                    pk_ap[O_WQ:O_WQ + D_MODEL * 128].rearrange("(c p m) -> p c m", p=128, m=128),
                )
                wk_sb = xwpool.tile([128, 8, 128], f16, tag="wk")
                nc.sync.dma_start(
                    wk_sb[:],
                    pk_ap[O_WK:O_WK + D_MODEL * 128].rearrange("(c p m) -> p c m", p=128, m=128),
                )
                wv_sb = xwpool.tile([128, 8, 128], f16, tag="wv")
                nc.gpsimd.dma_start(
                    wv_sb[:],
                    pk_ap[O_WV:O_WV + D_MODEL * 128].rearrange("(c p m) -> p c m", p=128, m=128),
                )
                # q path: 1/sqrt(dh) applied via activation scale (wqT ships raw f16)
                for dst, w_sb, b_sb, psc in (
                    (qT_sb, wq_sb, bq_sb, 0.125),
                    (kT_sb, wk_sb, bk_sb, 1.0),
                ):
                    for hh in range(2):
                        for nch in range(4):
                            ps = pA.tile([128, 512], f32, tag="proj")
                            for dc in range(8):
                                nc.tensor.matmul(
                                    ps[0:64, :],
                                    lhsT=w_sb[:, dc, hh * 64:(hh + 1) * 64],
                                    rhs=xT_sb[:, dc, nch * 512:(nch + 1) * 512],
                                    start=(dc == 0),
                                    stop=(dc == 7),
                                )
                            nc.scalar.activation(
                                dst[:, hh, nch * 512:(nch + 1) * 512], ps[0:64, :],
                                Act.Identity, bias=b_sb[:, hh:hh + 1], scale=psc,
                            )
                for kt in range(QT):
                    ps = pA.tile([128, 512], f32, tag="proj")
                    for dc in range(8):
                        nc.tensor.matmul(
                            ps[:, 0:128],
                            lhsT=xT_sb[:, dc, kt * 128:(kt + 1) * 128],
                            rhs=wv_sb[:, dc, :],
                            start=(dc == 0),
                            stop=(dc == 7),
                        )
                    nc.scalar.activation(v_sb[:, kt, :], ps[:, 0:128], Act.Identity)

            if _STAGE == 1:
                dbg = smpool.tile([64, 512], f32, tag="ot")
                nc.vector.tensor_copy(dbg[:], qT_sb[:, 0, 0:512])
                nc.sync.dma_start(out_d.ap()[0:64, 0:512], dbg[:])
            # ---- per-head state tiles (processed per half: 8 q-tiles) ----
            npool = ctx_npool = tc.tile_pool(name="nbig", bufs=1)
            npool = ctx_npool.__enter__()
            HQT = 8
            N32 = npool.tile([128, HQT, L], f32, tag="N32")
            bn6 = spool.tile([128, HQT, 6], f32, tag="bn6")
            bnagg = spool.tile([128, HQT, 2], f32, tag="bnagg")
            t_t = spool.tile([128, HQT], f32, tag="t")
            c_t = spool.tile([128, HQT], f32, tag="c")
            tp_t = spool.tile([128, HQT], f32, tag="tp")
            cp_t = spool.tile([128, HQT], f32, tag="cp")
            lo_t = spool.tile([128, HQT], f32, tag="lo")
            hi_t = spool.tile([128, HQT], f32, tag="hi")
            tb_t = spool.tile([128, HQT], f32, tag="tb")
            cb_t = spool.tile([128, HQT], f32, tag="cb")
            sd_t = spool.tile([128, HQT], f32, tag="sd")
            rsd_t = spool.tile([128, HQT], f32, tag="rsd")
            m_t = spool.tile([128, HQT], f32, tag="m")
            sel_t = spool.tile([128, HQT], f32, tag="sel")
            sk_t = spool.tile([128, HQT], f32, tag="sk")
            rd_t = spool.tile([128, HQT], f32, tag="rd")
            u_all = spool.tile([128, HQT, 8], f32, tag="u")
            oh_t = spool.tile([128, HQT, 8], f32, tag="oh")
            ohsel = spool.tile([128, HQT, 8], f32, tag="ohsel")
            tmp0 = spool.tile([128, HQT], f32, tag="tmp0")
            tmp1 = spool.tile([128, HQT], f32, tag="tmp1")
            tmp2 = spool.tile([128, HQT], f32, tag="tmp2")
            tmp3 = spool.tile([128, HQT], f32, tag="tmp3")
            mska = spool.tile([128, HQT], mybir.dt.uint8, tag="mska")
            mskb = spool.tile([128, HQT], mybir.dt.uint8, tag="mskb")

            SQ2PI_L = float(np.sqrt(2.0 * np.pi) / L)

            for h in range(2 if _STAGE >= 1.5 else 0):
              for half in range(2):
                J = list(range(half * HQT, (half + 1) * HQT))
                kk_ap = hc_sb[:, h, 0:1]
                tg_ap = hc_sb[:, h, 1:2]
                zq_ap = hc_sb[:, h, 2:3]
                mu_v = bnagg[:, :, 0]
                var_v = bnagg[:, :, 1]

                # B1: scores -> N = 16 - s ; subsample bn_stats
                with tc.tile_pool(name=f"pS{h}{half}", bufs=2, space="PSUM") as pS:
                    for jj, j in enumerate(J):
                        ps = pS.tile([128, L], f32, tag="sc")
                        for kc in range(4):
                            nc.tensor.matmul(
                                ps[:, kc * 512:(kc + 1) * 512],
                                lhsT=qT_sb[:, h, j * 128:(j + 1) * 128],
                                rhs=kT_sb[:, h, kc * 512:(kc + 1) * 512],
                                start=True, stop=True,
                            )
                        nc.scalar.activation(
                            N32[:, jj, :], ps[:], Act.Identity, bias=b16_sb[:, 0:1], scale=-1.0
                        )
                        nc.vector.bn_stats(bn6[:, jj, :], N32[:, jj, 0:512])

                if _STAGE < 2:
                    nc.sync.dma_start(out_d.ap()[0:128, :], N32[:, 0, 0:1024])
                    continue
                # B2: init
                for jj in range(HQT):
                    nc.vector.bn_aggr(bnagg[:, jj, :], bn6[:, jj:jj + 1, :])
                nc.scalar.activation(sd_t[:], var_v, Act.Sqrt)
                nc.vector.reciprocal(rsd_t[:], sd_t[:])
                nc.vector.tensor_scalar(t_t[:], sd_t[:], zq_ap, None, Alu.mult)
                nc.vector.tensor_tensor(t_t[:], t_t[:], mu_v, Alu.add)
                nc.vector.tensor_scalar(lo_t[:], sd_t[:], -4.0, None, Alu.mult)
                nc.vector.tensor_tensor(lo_t[:], lo_t[:], mu_v, Alu.add)
                nc.vector.tensor_scalar(hi_t[:], sd_t[:], 4.0, None, Alu.mult)
                nc.vector.tensor_tensor(hi_t[:], hi_t[:], mu_v, Alu.add)
                nc.vector.memset(tb_t[:], 30.0)
                nc.vector.memset(cb_t[:], 4096.0)

                if _STAGE < 2.5:
                    continue
                # B3: probe ladder (5 exact fused count passes)
                NPROBE = 5
                for it in range(NPROBE):
                    for jj in range(HQT):
                        scr = bigpool.tile([128, L], f32, tag="scr")
                        nc.vector.tensor_scalar(
                            scr[:], N32[:, jj, :], t_t[:, jj:jj + 1], None,
                            Alu.is_le, Alu.add, accum_out=c_t[:, jj:jj + 1],
                        )
                    # best-overcount select
                    nc.vector.tensor_scalar(tmp0[:], c_t[:], kk_ap, None, Alu.is_ge)
                    nc.vector.tensor_tensor(tmp1[:], c_t[:], cb_t[:], Alu.is_lt)
                    nc.vector.tensor_tensor(mska[:], tmp0[:], tmp1[:], Alu.mult)
                    nc.vector.copy_predicated(tb_t[:], mska[:], t_t[:])
                    nc.vector.copy_predicated(cb_t[:], mska[:], c_t[:])
                    # exact bracket update
                    nc.vector.tensor_scalar(mska[:], c_t[:], kk_ap, None, Alu.is_lt)
                    nc.vector.tensor_tensor(tmp2[:], lo_t[:], t_t[:], Alu.max)
                    nc.vector.copy_predicated(lo_t[:], mska[:], tmp2[:])
                    nc.vector.tensor_scalar(mskb[:], c_t[:], kk_ap, None, Alu.is_ge)
                    nc.vector.tensor_tensor(tmp2[:], hi_t[:], t_t[:], Alu.min)
                    nc.vector.copy_predicated(hi_t[:], mskb[:], tmp2[:])
                    if it == NPROBE - 1:
                        break
                    if it == 0:
                        nc.vector.tensor_copy(tp_t[:], t_t[:])
                        nc.vector.tensor_copy(cp_t[:], c_t[:])
                        nc.vector.tensor_tensor(tmp0[:], t_t[:], mu_v, Alu.subtract)
                        nc.vector.tensor_tensor(tmp0[:], tmp0[:], rsd_t[:], Alu.mult)
                        nc.scalar.activation(tmp1[:], tmp0[:], Act.Square, scale=0.7071067811865476)
                        nc.scalar.activation(tmp2[:], tmp1[:], Act.Exp)
                        nc.vector.tensor_tensor(tmp2[:], tmp2[:], sd_t[:], Alu.mult)
                        nc.vector.tensor_scalar(tmp2[:], tmp2[:], SQ2PI_L, None, Alu.mult)
                        nc.vector.tensor_scalar(tmp0[:], c_t[:], tg_ap, None, Alu.subtract)
                        nc.vector.tensor_tensor(tmp0[:], tmp0[:], tmp2[:], Alu.mult)
                        nc.vector.tensor_tensor(t_t[:], t_t[:], tmp0[:], Alu.subtract)
                    else:
                        nc.vector.tensor_tensor(tmp0[:], t_t[:], tp_t[:], Alu.subtract)
                        nc.vector.tensor_tensor(tmp1[:], c_t[:], cp_t[:], Alu.subtract)
                        nc.vector.reciprocal(tmp2[:], tmp0[:])
                        nc.vector.tensor_tensor(tmp1[:], tmp1[:], tmp2[:], Alu.mult)
                        nc.vector.tensor_scalar(tmp0[:], tmp1[:], 50.0, None, Alu.is_ge)
                        nc.vector.tensor_scalar(tmp2[:], tmp1[:], 1e6, None, Alu.is_le)
                        nc.vector.tensor_tensor(mska[:], tmp0[:], tmp2[:], Alu.mult)
                        nc.vector.memset(tmp2[:], 650.0)
                        nc.vector.copy_predicated(tmp2[:], mska[:], tmp1[:])
                        nc.vector.reciprocal(tmp3[:], tmp2[:])
                        nc.vector.tensor_copy(tp_t[:], t_t[:])
                        nc.vector.tensor_copy(cp_t[:], c_t[:])
                        nc.vector.tensor_scalar(tmp0[:], c_t[:], tg_ap, None, Alu.subtract)
                        nc.vector.tensor_tensor(tmp0[:], tmp0[:], tmp3[:], Alu.mult)
                        nc.vector.tensor_scalar(tmp1[:], tmp0[:], -1.3, None, Alu.mult)
                        nc.vector.tensor_scalar(tmp2[:], tmp3[:], 2.0, None, Alu.mult)
                        nc.vector.tensor_tensor(tmp1[:], tmp1[:], tmp2[:], Alu.max)
                        nc.vector.tensor_scalar(tmp1[:], tmp1[:], -1.0, None, Alu.mult)
                        nc.vector.tensor_scalar(mska[:], c_t[:], kk_ap, None, Alu.is_lt)
                        nc.vector.copy_predicated(tmp0[:], mska[:], tmp1[:])
                        nc.vector.tensor_tensor(t_t[:], t_t[:], tmp0[:], Alu.subtract)
                    nc.vector.tensor_tensor(t_t[:], t_t[:], hi_t[:], Alu.min)
                    nc.vector.tensor_tensor(mska[:], t_t[:], lo_t[:], Alu.is_le)
                    nc.vector.tensor_tensor(tmp1[:], lo_t[:], hi_t[:], Alu.add)
                    nc.vector.tensor_scalar(tmp1[:], tmp1[:], 0.5, None, Alu.mult)
                    nc.vector.copy_predicated(t_t[:], mska[:], tmp1[:])

                if _STAGE < 2.8:
                    continue
                # B4: exact snap, iterated. Each round moves the threshold to
                # the exact (m+1)-th largest kept score (m = min(excess, 7)),
                # which removes exactly m keys; cb is updated arithmetically so
                # NSNAP rounds handle excess <= 7*NSNAP without recount passes.
                NSNAP = 5
                for sr_i in range(NSNAP):
                    nc.vector.tensor_scalar(m_t[:], cb_t[:], kk_ap, None, Alu.subtract)
                    nc.vector.tensor_scalar(m_t[:], m_t[:], 7.0, 0.0, Alu.min, Alu.max)
                    for jj in range(HQT):
                        scr = bigpool.tile([128, L], f32, tag="scr")
                        nc.vector.scalar_tensor_tensor(
                            scr[:], N32[:, jj, :], tb_t[:, jj:jj + 1], N32[:, jj, :],
                            Alu.is_le, Alu.mult,
                        )
                        if _STAGE >= 2.82:
                            nc.vector.max(u_all[:, jj, :], scr[:])
                    if _STAGE < 2.83:
                        break
                    for jj in range(HQT):
                        nc.vector.tensor_scalar(
                            oh_t[:, jj, :], io8_sb[:], m_t[:, jj:jj + 1], None, Alu.is_equal
                        )
                    if _STAGE < 2.84:
                        break
                    for jj in range(HQT):
                        nc.vector.tensor_tensor(
                            ohsel[:, jj, :], oh_t[:, jj, :], u_all[:, jj, :], Alu.mult
                        )
                    for jj in range(HQT):
                        nc.vector.tensor_scalar(
                            oh_t[:, jj, :], ohsel[:, jj, :], 0.0, None,
                            Alu.add, Alu.add, accum_out=sel_t[:, jj:jj + 1],
                        )
                    nc.vector.tensor_scalar(mska[:], sel_t[:], 13.0, None, Alu.is_gt)
                    nc.vector.copy_predicated(tb_t[:], mska[:], sel_t[:])
                    nc.vector.tensor_tensor(tmp0[:], cb_t[:], m_t[:], Alu.subtract)
                    nc.vector.copy_predicated(cb_t[:], mska[:], tmp0[:])

                # B5: w, renormalized transpose, AV
                if _STAGE < 3:
                    continue
                with (
                    tc.tile_pool(name=f"pX{h}{half}", bufs=2, space="PSUM") as pX,
                    tc.tile_pool(name=f"pV{h}{half}", bufs=2, space="PSUM") as pV,
                ):
                    for jj, j in enumerate(J):
                        e_t = bigpool.tile([128, L], f32, tag="scr")
                        nc.scalar.activation(e_t[:], N32[:, jj, :], Act.Exp, bias=b16_sb[:, 0:1], scale=-1.0)
                        w_t = bigpool.tile([128, L], f32, tag="scr")
                        nc.vector.scalar_tensor_tensor(
                            w_t[:], N32[:, jj, :], tb_t[:, jj:jj + 1], e_t[:],
                            Alu.is_le, Alu.mult, accum_out=sk_t[:, jj:jj + 1],
                        )
                        nc.vector.reciprocal(rd_t[:, jj:jj + 1], sk_t[:, jj:jj + 1])
                        diag_t = smpool.tile([128, 128], f32, tag="diag")
                        nc.vector.tensor_scalar(
                            diag_t[:], io128_sb[:], pid_sb[:, 0:1], rd_t[:, jj:jj + 1],
                            Alu.is_equal, Alu.mult,
                        )
                        wT_t = wtpool.tile([128, QT, 128], f32, tag="wT")
                        for g in range(4):
                            psx = pX.tile([128, 512], f32, tag="x")
                            for s4 in range(4):
                                kc = g * 4 + s4
                                nc.tensor.matmul(
                                    psx[:, s4 * 128:(s4 + 1) * 128],
                                    lhsT=w_t[:, kc * 128:(kc + 1) * 128],
                                    rhs=diag_t[:],
                                    start=True, stop=True,
                                )
                            nc.scalar.activation(
                                wT_t[:, g * 4:(g + 1) * 4, :], psx[:], Act.Identity
                            )
                        psa = pV.tile([64, 128], f32, tag="av")
                        for kc in range(QT):
                            nc.tensor.matmul(
                                psa[:],
                                lhsT=v_sb[:, kc, h * 64:(h + 1) * 64],
                                rhs=wT_t[:, kc, :],
                                start=(kc == 0), stop=(kc == QT - 1),
                            )
                        nc.scalar.activation(yT_sb[:, h, j, :], psa[:], Act.Identity)

            ctx_npool.__exit__(None, None, None)

            # ---- phase E: output projection -> partial y, ReduceScatter, slice out ----
            with tc.tile_pool(name="pO", bufs=2, space="PSUM") as pO:
                for j in range(QT if _STAGE >= 3 else 0):
                    for oc in range(2):
                        pso = pO.tile([128, 512], f32, tag="o")
                        for hh in range(2):
                            nc.tensor.matmul(
                                pso[:],
                                lhsT=yT_sb[:, hh, j, :],
                                rhs=wo_sb[:, hh, oc * 512:(oc + 1) * 512],
                                start=(hh == 0), stop=(hh == 1),
                            )
                        o_t = smpool.tile([128, 512], f32, tag="ot")
                        nc.vector.tensor_copy(o_t[:], pso[:])
                        nc.sync.dma_start(
                            yb_b[j * 128:(j + 1) * 128, oc * 512:(oc + 1) * 512],
                            o_t[:],
                        )
            nc.gpsimd.collective_compute(
                "ReduceScatter",
                mybir.AluOpType.add,
                replica_groups=[list(range(NCORES))],
                ins=[yb_b.opt()],
                outs=[ys_b.opt()],
            )
            # f32 -> f16 on the way out (RS itself accumulates in f32)
            o32_t = smpool.tile([128, 2, D_MODEL], f32, tag="o32")
            nc.sync.dma_start(o32_t[:], ys_b[:].rearrange("(t p) m -> p t m", p=128))
            o16_t = smpool.tile([128, 2, D_MODEL], f16, tag="o16")
            nc.scalar.activation(o16_t[:], o32_t[:], Act.Identity)
            nc.sync.dma_start(out_d.ap().rearrange("(t p) m -> p t m", p=128), o16_t[:])
    nc.compile()
    return nc


def _host_prep(inputs):
    x = np.ascontiguousarray(np.asarray(inputs["x"], np.float32)[0])  # [L, D]
    Wq = np.asarray(inputs["Wq"], np.float32)
    Wk = np.asarray(inputs["Wk"], np.float32)
    Wv = np.asarray(inputs["Wv"], np.float32)
    Wo = np.asarray(inputs["Wo"], np.float32)
    bq = np.asarray(inputs["bq"], np.float32)
    bk = np.asarray(inputs["bk"], np.float32)
    bv = np.asarray(inputs["bv"], np.float32)
    bo = np.asarray(inputs["bo"], np.float32)
    ema = np.asarray(inputs["entropy_ema"], np.float32)
    thr = np.asarray(inputs["entropy_threshold"], np.float32)

    # k_keep exactly as the reference (fp32 sigmoid, truncation)
    sr = np.float32(MIN_SPARSITY) + np.float32(1.0 - MIN_SPARSITY) / (
        np.float32(1.0) + np.exp(-(ema - thr), dtype=np.float32)
    )
    kk = np.maximum(1, (np.float32(L) * (np.float32(1.0) - sr)).astype(np.int32))

    nd = NormalDist()
    scale = np.float32(1.0 / np.sqrt(D_HEAD))
    xT = np.ascontiguousarray(x.T)  # [D, L]

    LSH = L // NCORES
    xT16 = xT.astype(np.float16)
    in_maps = []
    for c in range(NCORES):
        rows = slice(c * 128, (c + 1) * 128)
        wqT = Wq[rows].T.astype(np.float16)
        wkT = Wk[rows].T.astype(np.float16)
        wvT = Wv[rows].T.astype(np.float16)
        woT = Wo[:, rows].T.astype(np.float16)
        bq2 = (bq[rows] * scale).reshape(128, 1)
        bk2 = bk[rows].reshape(128, 1)
        hconst = np.zeros((128, 2, 8), np.float32)
        for h in range(2):
            k_h = float(kk[2 * c + h])
            hconst[:, h, 0] = k_h
            hconst[:, h, 1] = k_h + 3.0
            hconst[:, h, 2] = np.float32(nd.inv_cdf(min(max(k_h / L, 1e-6), 1 - 1e-6)))
        xs = xT16[:, c * LSH:(c + 1) * LSH]
        pk16 = np.concatenate([
            xs.ravel(), wqT.ravel(), wkT.ravel(), wvT.ravel(), woT.ravel()
        ])
        pk32 = np.concatenate([bq2, bk2, hconst.reshape(128, 16)], axis=1)
        pk32 = np.ascontiguousarray(pk32, dtype=np.float32)
        in_maps.append({"pk16": pk16, "pk32": pk32})

    bo_eff = bo + bv @ Wo.T  # bv folded through the output projection
    return in_maps, bo_eff


def _enable_jax_cache():
    if _BUILD_CACHE.get("jax_cache_set"):
        return
    _BUILD_CACHE["jax_cache_set"] = True
    try:
        import jax
        jax.config.update("jax_compilation_cache_dir", "/tmp/jax_kernel_cc_cache")
        jax.config.update("jax_persistent_cache_min_entry_size_bytes", -1)
        jax.config.update("jax_persistent_cache_min_compile_time_secs", 0.0)
    except Exception:
        pass


def _make_cached_runner(nc):
    """Memoized variant of run_bass_kernel_spmd's axon path: identical
    lowering/execution (same _bass_exec_p custom call, same shard_map layout),
    but the jitted executable is built once and reused, skipping per-call
    retracing."""
    import jax
    import numpy as _np
    from jax.sharding import Mesh, PartitionSpec
    try:
        from jax import shard_map
        _shard_map = shard_map.shard_map if hasattr(shard_map, "shard_map") else shard_map
    except Exception:
        from jax.experimental.shard_map import shard_map as _shard_map
    import concourse.mybir as mybir
    from concourse.bass2jax import (
        _bass_exec_p,
        install_neuronx_cc_hook,
        partition_id_tensor,
    )

    install_neuronx_cc_hook()
    partition_name = nc.partition_id_tensor.name if nc.partition_id_tensor else None
    in_names, out_names, out_avals, out_shapes = [], [], [], []
    for alloc in nc.m.functions[0].allocations:
        if not isinstance(alloc, mybir.MemoryLocationSet):
            continue
        name = alloc.memorylocations[0].name
        if alloc.kind == "ExternalInput":
            if name != partition_name:
                in_names.append(name)
        elif alloc.kind == "ExternalOutput":
            out_names.append(name)
            shape = tuple(alloc.tensor_shape)
            dtype = mybir.dt.np(alloc.dtype)
            out_avals.append(jax.core.ShapedArray(shape, dtype))
            out_shapes.append((shape, dtype))
    n_params = len(in_names)
    in_names_full = list(in_names) + out_names + (
        [partition_name] if partition_name else []
    )
    donate = tuple(range(n_params, n_params + len(out_names)))

    def _body(*args):
        operands = list(args)
        if partition_name is not None:
            operands.append(partition_id_tensor())
        outs = _bass_exec_p.bind(
            *operands,
            out_avals=tuple(out_avals),
            in_names=tuple(in_names_full),
            out_names=tuple(out_names),
            lowering_input_output_aliases=(),
            sim_require_finite=True,
            sim_require_nnan=True,
            nc=nc,
        )
        return tuple(outs)

    devices = jax.devices()[:NCORES]
    mesh = Mesh(_np.asarray(devices), ("core",))
    in_specs = (PartitionSpec("core"),) * (n_params + len(out_names))
    out_specs = (PartitionSpec("core"),) * len(out_names)
    sharded = jax.jit(
        _shard_map(
            _body, mesh=mesh, in_specs=in_specs, out_specs=out_specs, check_rep=False
        ),
        donate_argnums=donate,
        keep_unused=True,
    )

    def run(in_maps):
        concat_in = [
            _np.concatenate([_np.asarray(m[name]) for m in in_maps], axis=0)
            for name in in_names
        ]
        concat_zeros = [
            _np.zeros((NCORES * s[0], *s[1:]), dt) for (s, dt) in out_shapes
        ]
        out_arrs = sharded(*concat_in, *concat_zeros)
        return [
            {
                name: _np.asarray(out_arrs[i]).reshape(NCORES, *out_shapes[i][0])[c]
                for i, name in enumerate(out_names)
            }
            for c in range(NCORES)
        ]

    return run


def _run_spmd(nc, in_maps):
    runner = _BUILD_CACHE.get("runner", "unset")
    if runner == "unset":
        try:
            runner = _make_cached_runner(nc)
        except Exception:
            runner = None
        _BUILD_CACHE["runner"] = runner
    if runner is not None:
        try:
            return runner(in_maps)
        except Exception:
            _BUILD_CACHE["runner"] = None
    from concourse.bass_utils import run_bass_kernel_spmd

    res = run_bass_kernel_spmd(nc, in_maps, list(range(NCORES)))
    return res.results


def kernel(**inputs):
    _enable_jax_cache()
    if "nc" not in _BUILD_CACHE:
        _BUILD_CACHE["nc"] = _build_nc()
    nc = _BUILD_CACHE["nc"]

    in_maps, bo_eff = _host_prep(inputs)
    results = _run_spmd(nc, in_maps)
    out = np.concatenate(
        [results[c]["outp"].astype(np.float32) for c in range(NCORES)], axis=0
    )
    out = out + bo_eff[None, :]
    return out[None].astype(np.float32)

